# revision 35
# baseline (speedup 1.0000x reference)
"""Trainium2 (8 NeuronCores) kernel for AudioCodebook.find_nearest_codes.

Reference computation (see problem):
    d2 = ||x||^2 - 2 x.c + ||c||^2  over latents [512, 32000] x codebook [1024, 32000]
    indices = argmin_k d2            [512] int32
    quantized = codebook[indices]    [512, 8, 250, 16]
    min_distances = sqrt(d2_min)     [512]
    new_usage = usage + hist(indices)[1024]

Distribution: 2 batch-groups x 4 code-shards over 8 cores.  Each core
streams its (256-row, 256-code) block of the -d2 matmul in float32r
(contraction F=32000 on partitions, 250 chunks of 128), folds the
host-precomputed norms in with one tiny fp32 rank-2 matmul, takes the
per-shard argmin with vector max/max_index, AllGathers the (val, idx)
candidates across all 8 cores, combines locally, and then each core
indirect-DMA-gathers the winning codebook rows for its 64 output rows.
"""
import sys

sys.path.insert(0, "/opt/trn_rl_repo")
import numpy as np

B, K, F = 512, 1024, 32000
LATENT_SHAPE = (8, 250, 16)
NCORES = 8
R, C = 2, 4              # batch groups x code shards
BG = B // R              # 256 rows per batch group
KS = K // C              # 256 codes per shard
MT = BG // 128           # 2 M-tiles per group
P = 128
NCH = F // P             # 250 contraction chunks
FB = 10                  # chunks per DMA block
NB = NCH // FB           # 25 blocks
SEG = 2000               # gather segment width (per cb4 row)
NSEG = 8                 # segments per half-row
CB4_ROWS = K * 16        # codebook viewed as [16384, 2000]
ROWS_PER_CORE = B // NCORES  # 64

_BUILT = {}


def _build():
    from concourse import bass, bacc, mybir
    import concourse.tile as tile

    dt = mybir.dt
    nc = bacc.Bacc("TRN2", target_bir_lowering=False, debug=False, num_devices=NCORES)

    lf = nc.dram_tensor("lf", [P, NCH * BG], dt.float32r, kind="ExternalInput")
    cf = nc.dram_tensor("cf", [P, NCH * KS], dt.float32r, kind="ExternalInput")
    normx = nc.dram_tensor("normx", [P, MT], dt.float32, kind="ExternalInput")
    normc = nc.dram_tensor("normc", [P, KS], dt.float32, kind="ExternalInput")
    usage = nc.dram_tensor("usage", [1, K], dt.float32, kind="ExternalInput")
    myrows2 = nc.dram_tensor("myrows2", [P, 1], dt.int32, kind="ExternalInput")
    parity4 = nc.dram_tensor("parity4", [P, 1], dt.float32, kind="ExternalInput")
    sbias = nc.dram_tensor("sbias", [P, 16], dt.float32, kind="ExternalInput")
    codebook = nc.dram_tensor("codebook", [CB4_ROWS, SEG], dt.float32, kind="ExternalInput")

    out_q = nc.dram_tensor("out_q", [P, NSEG * SEG], dt.float32, kind="ExternalOutput")
    out_idx = nc.dram_tensor("out_idx", [P, 4], dt.int32, kind="ExternalOutput")
    out_md = nc.dram_tensor("out_md", [P, 4], dt.float32, kind="ExternalOutput")
    out_usage = nc.dram_tensor("out_usage", [1, K], dt.float32, kind="ExternalOutput")

    with tile.TileContext(nc) as tc:
        with (
            tc.tile_pool(name="stream", bufs=4) as spool,
            tc.tile_pool(name="misc", bufs=1) as mpool,
            tc.tile_pool(name="psum", bufs=1, space="PSUM") as ppool,
            tc.tile_pool(name="gat", bufs=6) as gpool,
            tc.tile_pool(name="dram", bufs=1, space="DRAM") as dpool,
        ):
            acc = [
                ppool.tile([P, KS], dt.float32, space="PSUM", name=f"acc{t}")
                for t in range(MT)
            ]

            # ---- streaming -d2 matmul: PSUM += 2*x.c (norms folded in later)
            # small first blocks so the PE starts as soon as possible
            blocks = [2, 8] + [FB] * (NB - 1)
            assert sum(blocks) == NCH
            ch0 = 0
            for blk, fb in enumerate(blocks):
                lf_sl = spool.tile([P, FB * BG], dt.float32r, tag="lf")
                cf_sl = spool.tile([P, FB * KS], dt.float32r, tag="cf")
                nc.sync.dma_start(
                    lf_sl[:, :fb * BG], lf[:, ch0 * BG:(ch0 + fb) * BG]
                )
                nc.sync.dma_start(
                    cf_sl[:, :fb * KS], cf[:, ch0 * KS:(ch0 + fb) * KS]
                )
                if blk == 0:
                    # warm up ncfw while the stream runs so the real
                    # AllGather at the end doesn't pay the wakeup cost
                    warm_in = dpool.tile([P, 1], dt.float32)
                    warm_out = dpool.tile([NCORES * P, 1], dt.float32)
                    nc.gpsimd.collective_compute(
                        "AllGather", mybir.AluOpType.bypass,
                        replica_groups=[list(range(NCORES))],
                        ins=[warm_in.opt()], outs=[warm_out.opt()],
                    )
                    # warm the SWDGE dynamic-DMA path as well
                    warm_g = mpool.tile([P, 1], dt.float32)
                    warm_ix = mpool.tile([P, 1], dt.int32)
                    nc.vector.memset(warm_ix[:], 0)
                    nc.gpsimd.indirect_dma_start(
                        out=warm_g[:], out_offset=None,
                        in_=warm_in[:],
                        in_offset=bass.IndirectOffsetOnAxis(ap=warm_ix[:, :1], axis=0),
                    )
                for ch in range(fb):
                    for t in range(MT):
                        nc.tensor.matmul(
                            acc[t][:],
                            lf_sl[:, ch * BG + t * 128: ch * BG + (t + 1) * 128],
                            cf_sl[:, ch * KS:(ch + 1) * KS],
                            start=(ch0 + ch == 0),
                            stop=(ch0 + ch == NCH - 1),
                        )
                ch0 += fb

            # ---- small inputs for the tail (DMAs fill stream gaps)
            mr = mpool.tile([P, 1], dt.int32)
            par = mpool.tile([P, 1], dt.float32)
            sb_t = mpool.tile([P, 16], dt.float32)
            nc.sync.dma_start(mr[:], myrows2[:])
            nc.sync.dma_start(par[:], parity4[:])
            nc.sync.dma_start(sb_t[:], sbias[:])
            iota_i = mpool.tile([P, K], dt.int32)
            nc.gpsimd.iota(iota_i[:], pattern=[[1, K]], base=0, channel_multiplier=0)

            # ---- fold in -(x2) and -(c2) with exact fp32 adds on DVE:
            # neg = 2*x.c - x2[m] - c2[j]  (= -d2)
            nx = mpool.tile([P, MT], dt.float32)
            ncn = mpool.tile([P, KS], dt.float32)
            nc.sync.dma_start(nx[:], normx[:])
            nc.sync.dma_start(ncn[:], normc[:])
            neg = mpool.tile([P, MT * KS], dt.float32)
            for t in range(MT):
                nc.vector.tensor_tensor(
                    neg[:, t * KS:(t + 1) * KS], acc[t][:], ncn[:],
                    op=mybir.AluOpType.add,
                )
                nc.vector.tensor_tensor(
                    neg[:, t * KS:(t + 1) * KS], neg[:, t * KS:(t + 1) * KS],
                    nx[:, t:t + 1].to_broadcast([P, KS]),
                    op=mybir.AluOpType.add,
                )
            mx = mpool.tile([P, MT * 8], dt.float32)
            mi = mpool.tile([P, MT * 8], dt.uint32)
            for t in range(MT):
                nc.vector.max(out=mx[:, t * 8:(t + 1) * 8], in_=neg[:, t * KS:(t + 1) * KS])
                nc.vector.max_index(
                    out=mi[:, t * 8:(t + 1) * 8],
                    in_max=mx[:, t * 8:(t + 1) * 8],
                    in_values=neg[:, t * KS:(t + 1) * KS],
                )

            # ---- AllGather (v_t0, v_t1, i_t0, i_t1) candidates from all 8 cores
            pay = mpool.tile([P, 2 * MT], dt.float32)
            for t in range(MT):
                nc.vector.tensor_copy(pay[:, t:t + 1], mx[:, t * 8:t * 8 + 1])
                nc.vector.tensor_copy(pay[:, 2 + t:3 + t], mi[:, t * 8:t * 8 + 1])
            ag_in = dpool.tile([P, 2 * MT], dt.float32)
            ag_out = dpool.tile([NCORES * P, 2 * MT], dt.float32)
            nc.sync.dma_start(ag_in[:], pay[:])
            nc.gpsimd.collective_compute(
                "AllGather", mybir.AluOpType.bypass,
                replica_groups=[list(range(NCORES))],
                ins=[ag_in.opt()], outs=[ag_out.opt()],
            )
            # vals/idxs [128, 16] with col = s*4 + (g*2 + t); ag_out element
            # (q=4g+s, p, vc*2+t) lives at dram offset (4g+s)*512 + p*4 + vc*2 + t
            ag5 = ag_out.rearrange("(g s p) (vc t) -> p s g vc t", g=2, s=4, vc=2)
            vals = mpool.tile([P, 16], dt.float32)
            idxs = mpool.tile([P, 16], dt.float32)
            vals4 = vals[:].rearrange("p (s g t) -> p s g t", s=4, g=2)
            idxs4 = idxs[:].rearrange("p (s g t) -> p s g t", s=4, g=2)
            for g in range(2):
                nc.sync.dma_start(vals4[:, :, g, :].opt(), ag5[:, :, g, 0, :].opt())
                nc.sync.dma_start(idxs4[:, :, g, :].opt(), ag5[:, :, g, 1, :].opt())
            # global idx = local + s*256
            nc.vector.tensor_tensor(idxs[:], idxs[:], sb_t[:], op=mybir.AluOpType.add)

            # ---- vectorized 4-way combine over shards (all 4 h at once)
            # ties: is_ge prefers the lower shard, matching argmin-first.
            v01 = mpool.tile([P, 4], dt.float32)
            v23 = mpool.tile([P, 4], dt.float32)
            i01 = mpool.tile([P, 4], dt.float32)
            i23 = mpool.tile([P, 4], dt.float32)
            ge = mpool.tile([P, 4], dt.uint32)
            bestv = mpool.tile([P, 4], dt.float32)
            besti = mpool.tile([P, 4], dt.float32)
            nc.vector.tensor_tensor(ge[:], vals[:, 0:4], vals[:, 4:8], op=mybir.AluOpType.is_ge)
            nc.vector.tensor_tensor(v01[:], vals[:, 0:4], vals[:, 4:8], op=mybir.AluOpType.max)
            nc.vector.tensor_copy(i01[:], idxs[:, 4:8])
            nc.vector.copy_predicated(i01[:], ge[:], idxs[:, 0:4])
            nc.vector.tensor_tensor(ge[:], vals[:, 8:12], vals[:, 12:16], op=mybir.AluOpType.is_ge)
            nc.vector.tensor_tensor(v23[:], vals[:, 8:12], vals[:, 12:16], op=mybir.AluOpType.max)
            nc.vector.tensor_copy(i23[:], idxs[:, 12:16])
            nc.vector.copy_predicated(i23[:], ge[:], idxs[:, 8:12])
            nc.vector.tensor_tensor(ge[:], v01[:], v23[:], op=mybir.AluOpType.is_ge)
            nc.vector.tensor_tensor(bestv[:], v01[:], v23[:], op=mybir.AluOpType.max)
            nc.vector.tensor_copy(besti[:], i23[:])
            nc.vector.copy_predicated(besti[:], ge[:], i01[:])

            # ---- critical path: gather this core's 64 quantized rows
            idx_scr = dpool.tile([B, 1], dt.float32)
            nc.sync.dma_start(idx_scr.rearrange("(p h) o -> p (h o)", p=P), besti[:])
            gidxf = mpool.tile([P, 1], dt.float32)
            nc.gpsimd.indirect_dma_start(
                out=gidxf[:], out_offset=None,
                in_=idx_scr[:],
                in_offset=bass.IndirectOffsetOnAxis(ap=mr[:, :1], axis=0),
            )
            base_f = mpool.tile([P, 1], dt.float32)
            nc.vector.tensor_scalar(base_f[:], gidxf[:], 16.0, scalar2=None, op0=mybir.AluOpType.mult)
            nc.vector.tensor_tensor(base_f[:], base_f[:], par[:], op=mybir.AluOpType.add)
            idx8 = mpool.tile([P, NSEG], dt.int32)
            for cseg in range(NSEG):
                nc.vector.tensor_scalar(
                    idx8[:, cseg:cseg + 1], base_f[:], float(cseg),
                    scalar2=None, op0=mybir.AluOpType.add,
                )
            for cseg in range(NSEG):
                gt = gpool.tile([P, SEG], dt.float32, tag="gt")
                nc.gpsimd.indirect_dma_start(
                    out=gt[:], out_offset=None,
                    in_=codebook[:],
                    in_offset=bass.IndirectOffsetOnAxis(ap=idx8[:, cseg:cseg + 1], axis=0),
                )
                nc.sync.dma_start(out_q[:, cseg * SEG:(cseg + 1) * SEG], gt[:])

            # ---- indices / min_distances (off critical path)
            idx_i = mpool.tile([P, 4], dt.int32)
            nc.vector.tensor_copy(idx_i[:], besti[:])
            nc.sync.dma_start(out_idx[:], idx_i[:])
            md = mpool.tile([P, 4], dt.float32)
            nc.scalar.activation(
                md[:], bestv[:], mybir.ActivationFunctionType.Sqrt, scale=-1.0,
            )
            nc.sync.dma_start(out_md[:], md[:])

            # ---- usage histogram (identical on every core, off critical path)
            idxf = mpool.tile([P, 4], dt.float32)
            nc.vector.tensor_copy(idxf[:], idx_i[:])
            iota_f = mpool.tile([P, K], dt.float32)
            nc.vector.tensor_copy(iota_f[:], iota_i[:])
            eqacc = mpool.tile([P, K], dt.float32)
            eqtmp = mpool.tile([P, K], dt.float32)
            for h in range(4):
                dst = eqacc if h == 0 else eqtmp
                nc.vector.tensor_tensor(
                    dst[:], idxf[:, h:h + 1].to_broadcast([P, K]), iota_f[:],
                    op=mybir.AluOpType.is_equal,
                )
                if h > 0:
                    nc.vector.tensor_tensor(eqacc[:], eqacc[:], eqtmp[:], op=mybir.AluOpType.add)
            ones1 = mpool.tile([P, 1], dt.float32)
            nc.vector.memset(ones1[:], 1.0)
            hist_ps = ppool.tile([1, K], dt.float32, space="PSUM")
            for half in range(2):
                nc.tensor.matmul(
                    hist_ps[:, half * 512:(half + 1) * 512],
                    ones1[:], eqacc[:, half * 512:(half + 1) * 512],
                    start=True, stop=True,
                )
            us = mpool.tile([1, K], dt.float32)
            nc.sync.dma_start(us[:], usage[:])
            nc.vector.tensor_tensor(us[:], us[:], hist_ps[:], op=mybir.AluOpType.add)
            nc.sync.dma_start(out_usage[:], us[:])

    nc.compile()
    return nc


def _get_nc():
    if "nc" not in _BUILT:
        _BUILT["nc"] = _build()
    return _BUILT["nc"]


def kernel(latents, codebook, usage_count):
    from concourse.bass_utils import run_bass_kernel_spmd

    latents = np.ascontiguousarray(np.asarray(latents, dtype=np.float32))
    codebook = np.ascontiguousarray(np.asarray(codebook, dtype=np.float32))
    usage_count = np.asarray(usage_count, dtype=np.float32)

    lat = latents.reshape(B, F)
    cb = codebook.reshape(K, F)

    # host-side norms (fp64 for accuracy, stored negated as fp32)
    x2 = (lat.astype(np.float64) ** 2).sum(axis=1)
    c2 = (cb.astype(np.float64) ** 2).sum(axis=1)

    # rearranged streaming operands: [128 p, 250 ch, cols], f index = ch*128 + p
    def rearr(mat2d, scale):
        # mat2d [rows, F] -> [P, NCH*rows] with layout [p, ch, row]
        m = (scale * mat2d).astype(np.float32)          # [rows, F]
        t = m.T.reshape(NCH, P, m.shape[0])             # [ch, p, row]
        return np.ascontiguousarray(t.transpose(1, 0, 2)).reshape(P, NCH * m.shape[0])

    lf_g = [rearr(lat[g * BG:(g + 1) * BG], 2.0) for g in range(R)]
    cf_s = [rearr(cb[s * KS:(s + 1) * KS], 1.0) for s in range(C)]

    cb4 = codebook.reshape(CB4_ROWS, SEG)
    usage_in = usage_count.reshape(1, K)
    parity = ((np.arange(P) % 2) * 8.0).astype(np.float32).reshape(P, 1)
    sbias_v = np.ascontiguousarray(
        np.broadcast_to(
            (np.arange(16) // 4 * KS).astype(np.float32), (P, 16)
        )
    )

    in_maps = []
    for c in range(NCORES):
        g, s = divmod(c, C)
        nx = np.empty((P, MT), np.float32)
        for t in range(MT):
            nx[:, t] = -x2[g * BG + t * 128:g * BG + (t + 1) * 128]
        ncn = np.ascontiguousarray(
            np.broadcast_to(-c2[s * KS:(s + 1) * KS].astype(np.float32), (P, KS))
        )
        rows = c * ROWS_PER_CORE + (np.arange(P) // 2)   # global row per partition (dup'd)
        mr = ((rows % P) * 4 + rows // P).astype(np.int32).reshape(P, 1)
        in_maps.append(dict(
            lf=lf_g[g], cf=cf_s[s], normx=nx, normc=ncn,
            usage=usage_in, myrows2=mr, parity4=parity, sbias=sbias_v,
            codebook=cb4,
        ))

    nc = _get_nc()
    res = run_bass_kernel_spmd(
        nc, in_maps, core_ids=list(range(NCORES)), **_BUILT.get("run_kwargs", {})
    )
    _BUILT["last_result"] = res
    r0 = res.results[0]

    indices = r0["out_idx"].T.reshape(B).astype(np.int32)
    min_distances = r0["out_md"].T.reshape(B).astype(np.float32)
    new_usage = r0["out_usage"].reshape(K).astype(np.float32)
    quantized = np.concatenate(
        [res.results[c]["out_q"].reshape(ROWS_PER_CORE, F) for c in range(NCORES)], axis=0
    ).reshape((B,) + LATENT_SHAPE)
    return indices, quantized, min_distances, new_usage


# revision 39
# speedup vs baseline: 1.1059x; 1.1059x over previous
"""Trainium2 (8 NeuronCores) kernel for AudioCodebook.find_nearest_codes.

Reference computation (see problem):
    d2 = ||x||^2 - 2 x.c + ||c||^2  over latents [512, 32000] x codebook [1024, 32000]
    indices = argmin_k d2            [512] int32
    quantized = codebook[indices]    [512, 8, 250, 16]
    min_distances = sqrt(d2_min)     [512]
    new_usage = usage + hist(indices)[1024]

Distribution: 2 F-halves x 4 code-shards over 8 cores.  Each core
streams all 512 latent rows over its 16000-element F-half plus its
256-code shard of the codebook (49MB/core, read exactly once chip-wide),
accumulating partial 2*x.c in PSUM with float32r matmuls.  Host-computed
-x2/2, -c2/2 are folded in with exact fp32 DVE adds, a pairwise
ReduceScatter sums the two F-half partials (each partner keeps the full
-d2 for its own 256 rows), vector max/max_index takes the per-shard
argmin, one 8-core AllGather shares the (val, idx) candidates, a
vectorized 4-way combine picks the global winner per row, and each core
indirect-DMA-gathers the winning codebook rows for its 64 output rows.
Dummy warm-up collectives early in the stream hide the ncfw wakeup cost.
"""
import sys

sys.path.insert(0, "/opt/trn_rl_repo")
import numpy as np

B, K, F = 512, 1024, 32000
LATENT_SHAPE = (8, 250, 16)
NCORES = 8
FH = 2                   # contraction (F) halves
C = 4                    # code shards
KS = K // C              # 256 codes per shard
MT = 4                   # row tiles (all 512 rows on every core)
P = 128
FHALF = F // FH          # 16000
NCH = FHALF // P         # 125 contraction chunks per core
FB = 10                  # chunks per DMA block
SEG = 4000               # gather segment width (per cb4 row)
NSEG = 4                 # segments per half-row
CB4_ROWS = K * 8         # codebook viewed as [8192, 4000]
ROWS_PER_CORE = B // NCORES  # 64
PAIRS = [[s, s + 4] for s in range(C)]   # F-half partner cores

_BUILT = {}


def _build():
    from concourse import bass, bacc, mybir
    import concourse.tile as tile

    dt = mybir.dt
    nc = bacc.Bacc("TRN2", target_bir_lowering=False, debug=False, num_devices=NCORES)

    lf = nc.dram_tensor("lf", [P, NCH * B], dt.float32r, kind="ExternalInput")
    cf = nc.dram_tensor("cf", [P, NCH * KS], dt.float32r, kind="ExternalInput")
    normx = nc.dram_tensor("normx", [P, MT], dt.float32, kind="ExternalInput")
    normc = nc.dram_tensor("normc", [P, KS], dt.float32, kind="ExternalInput")
    usage = nc.dram_tensor("usage", [1, K], dt.float32, kind="ExternalInput")
    myrows2 = nc.dram_tensor("myrows2", [P, 1], dt.int32, kind="ExternalInput")
    parity4 = nc.dram_tensor("parity4", [P, 1], dt.float32, kind="ExternalInput")
    sbias = nc.dram_tensor("sbias", [P, 16], dt.float32, kind="ExternalInput")
    codebook = nc.dram_tensor("codebook", [CB4_ROWS, SEG], dt.float32, kind="ExternalInput")

    out_q = nc.dram_tensor("out_q", [P, NSEG * SEG], dt.float32, kind="ExternalOutput")
    out_idx = nc.dram_tensor("out_idx", [P, 4], dt.int32, kind="ExternalOutput")
    out_md = nc.dram_tensor("out_md", [P, 4], dt.float32, kind="ExternalOutput")
    out_usage = nc.dram_tensor("out_usage", [1, K], dt.float32, kind="ExternalOutput")

    with tile.TileContext(nc) as tc:
        with (
            tc.tile_pool(name="stream", bufs=3) as spool,
            tc.tile_pool(name="misc", bufs=1) as mpool,
            tc.tile_pool(name="psum", bufs=1, space="PSUM") as ppool,
            tc.tile_pool(name="gat", bufs=4) as gpool,
            tc.tile_pool(name="dram", bufs=1, space="DRAM") as dpool,
        ):
            acc = [
                ppool.tile([P, KS], dt.float32, space="PSUM", name=f"acc{t}")
                for t in range(MT)
            ]

            # ---- streaming partial -d2 matmul over this core's F-half
            # small first blocks so the PE starts as soon as possible
            blocks = [2, 8] + [FB] * 11 + [5]
            assert sum(blocks) == NCH
            ch0 = 0
            for blk, fb in enumerate(blocks):
                lf_sl = spool.tile([P, FB * B], dt.float32r, tag="lf")
                cf_sl = spool.tile([P, FB * KS], dt.float32r, tag="cf")
                nc.sync.dma_start(
                    lf_sl[:, :fb * B], lf[:, ch0 * B:(ch0 + fb) * B]
                )
                nc.sync.dma_start(
                    cf_sl[:, :fb * KS], cf[:, ch0 * KS:(ch0 + fb) * KS]
                )
                if blk == 0:
                    # warm up ncfw while the stream runs (AR then AG --
                    # same order as the real collectives at the tail)
                    warm_in = dpool.tile([P, 1], dt.float32)
                    warm_rs = dpool.tile([P // 2, 1], dt.float32)
                    warm_out = dpool.tile([NCORES * P, 1], dt.float32)
                    nc.gpsimd.collective_compute(
                        "ReduceScatter", mybir.AluOpType.add,
                        replica_groups=PAIRS,
                        ins=[warm_in.opt()], outs=[warm_rs.opt()],
                    )
                    nc.gpsimd.collective_compute(
                        "AllGather", mybir.AluOpType.bypass,
                        replica_groups=[list(range(NCORES))],
                        ins=[warm_in.opt()], outs=[warm_out.opt()],
                    )
                    # warm the SWDGE dynamic-DMA path as well
                    warm_g = mpool.tile([P, 1], dt.float32)
                    warm_ix = mpool.tile([P, 1], dt.int32)
                    nc.vector.memset(warm_ix[:], 0)
                    nc.gpsimd.indirect_dma_start(
                        out=warm_g[:], out_offset=None,
                        in_=warm_in[:],
                        in_offset=bass.IndirectOffsetOnAxis(ap=warm_ix[:, :1], axis=0),
                    )
                for ch in range(fb):
                    for t in range(MT):
                        nc.tensor.matmul(
                            acc[t][:],
                            lf_sl[:, ch * B + t * 128: ch * B + (t + 1) * 128],
                            cf_sl[:, ch * KS:(ch + 1) * KS],
                            start=(ch0 + ch == 0),
                            stop=(ch0 + ch == NCH - 1),
                        )
                ch0 += fb

            # ---- small inputs for the tail (DMAs fill stream gaps)
            mr = mpool.tile([P, 1], dt.int32)
            par = mpool.tile([P, 1], dt.float32)
            sb_t = mpool.tile([P, 16], dt.float32)
            nc.sync.dma_start(mr[:], myrows2[:])
            nc.sync.dma_start(par[:], parity4[:])
            nc.sync.dma_start(sb_t[:], sbias[:])
            iota_i = mpool.tile([P, K], dt.int32)
            nc.gpsimd.iota(iota_i[:], pattern=[[1, K]], base=0, channel_multiplier=0)

            # ---- fold in -(x2) and -(c2) with exact fp32 adds on DVE:
            # neg = 2*x.c - x2[m] - c2[j]  (= -d2)
            nx = mpool.tile([P, MT], dt.float32)
            ncn = mpool.tile([P, KS], dt.float32)
            nc.sync.dma_start(nx[:], normx[:])
            nc.sync.dma_start(ncn[:], normc[:])
            neg = mpool.tile([P, MT * KS], dt.float32)
            for t in range(MT):
                nc.vector.tensor_tensor(
                    neg[:, t * KS:(t + 1) * KS], acc[t][:], ncn[:],
                    op=mybir.AluOpType.add,
                )
                nc.vector.tensor_tensor(
                    neg[:, t * KS:(t + 1) * KS], neg[:, t * KS:(t + 1) * KS],
                    nx[:, t:t + 1].to_broadcast([P, KS]),
                    op=mybir.AluOpType.add,
                )
            # ---- ReduceScatter the two F-half partials: each pair rank
            # gets the full d2 for its own 256 rows (fh ordering matches
            # the [s, s+4] group listing)
            dr_in = dpool.tile([B, KS], dt.float32)
            dr_out = dpool.tile([B // 2, KS], dt.float32)
            nc.sync.dma_start(
                dr_in.rearrange("(t p) c -> p t c", t=MT),
                neg[:].rearrange("p (t c) -> p t c", t=MT),
            )
            nc.gpsimd.collective_compute(
                "ReduceScatter", mybir.AluOpType.add,
                replica_groups=PAIRS,
                ins=[dr_in.opt()], outs=[dr_out.opt()],
            )
            LMT = 2
            negf = mpool.tile([P, LMT * KS], dt.float32)
            nc.sync.dma_start(
                negf[:].rearrange("p (t c) -> p t c", t=LMT),
                dr_out.rearrange("(t p) c -> p t c", t=LMT),
            )
            mx = mpool.tile([P, LMT * 8], dt.float32)
            mi = mpool.tile([P, LMT * 8], dt.uint32)
            for t in range(LMT):
                nc.vector.max(out=mx[:, t * 8:(t + 1) * 8], in_=negf[:, t * KS:(t + 1) * KS])
                nc.vector.max_index(
                    out=mi[:, t * 8:(t + 1) * 8],
                    in_max=mx[:, t * 8:(t + 1) * 8],
                    in_values=negf[:, t * KS:(t + 1) * KS],
                )

            # ---- AllGather (v_t0, v_t1, i_t0, i_t1) candidates
            pay = mpool.tile([P, 2 * LMT], dt.float32)
            for t in range(LMT):
                nc.vector.tensor_copy(pay[:, t:t + 1], mx[:, t * 8:t * 8 + 1])
                nc.vector.tensor_copy(pay[:, 2 + t:3 + t], mi[:, t * 8:t * 8 + 1])
            ag_in = dpool.tile([P, 2 * LMT], dt.float32)
            ag_out = dpool.tile([NCORES * P, 2 * LMT], dt.float32)
            nc.sync.dma_start(ag_in[:], pay[:])
            nc.gpsimd.collective_compute(
                "AllGather", mybir.AluOpType.bypass,
                replica_groups=[list(range(NCORES))],
                ins=[ag_in.opt()], outs=[ag_out.opt()],
            )
            # vals/idxs [128, 16] with col = s*4 + (g*2 + t), g = fh of the
            # contributing core (block q = g*4 + s)
            ag5 = ag_out.rearrange("(g s p) (vc t) -> p s g vc t", g=2, s=4, vc=2)
            vals = mpool.tile([P, 16], dt.float32)
            idxs = mpool.tile([P, 16], dt.float32)
            vals4 = vals[:].rearrange("p (s g t) -> p s g t", s=4, g=2)
            idxs4 = idxs[:].rearrange("p (s g t) -> p s g t", s=4, g=2)
            for g in range(2):
                nc.sync.dma_start(vals4[:, :, g, :].opt(), ag5[:, :, g, 0, :].opt())
                nc.sync.dma_start(idxs4[:, :, g, :].opt(), ag5[:, :, g, 1, :].opt())
            # global idx = local + s*256
            nc.vector.tensor_tensor(idxs[:], idxs[:], sb_t[:], op=mybir.AluOpType.add)

            # ---- vectorized 4-way combine over shards (all 4 h at once)
            # ties: is_ge prefers the lower shard, matching argmin-first.
            v01 = mpool.tile([P, 4], dt.float32)
            v23 = mpool.tile([P, 4], dt.float32)
            i01 = mpool.tile([P, 4], dt.float32)
            i23 = mpool.tile([P, 4], dt.float32)
            ge = mpool.tile([P, 4], dt.uint32)
            bestv = mpool.tile([P, 4], dt.float32)
            besti = mpool.tile([P, 4], dt.float32)
            nc.vector.tensor_tensor(ge[:], vals[:, 0:4], vals[:, 4:8], op=mybir.AluOpType.is_ge)
            nc.vector.tensor_tensor(v01[:], vals[:, 0:4], vals[:, 4:8], op=mybir.AluOpType.max)
            nc.vector.tensor_copy(i01[:], idxs[:, 4:8])
            nc.vector.copy_predicated(i01[:], ge[:], idxs[:, 0:4])
            nc.vector.tensor_tensor(ge[:], vals[:, 8:12], vals[:, 12:16], op=mybir.AluOpType.is_ge)
            nc.vector.tensor_tensor(v23[:], vals[:, 8:12], vals[:, 12:16], op=mybir.AluOpType.max)
            nc.vector.tensor_copy(i23[:], idxs[:, 12:16])
            nc.vector.copy_predicated(i23[:], ge[:], idxs[:, 8:12])
            nc.vector.tensor_tensor(ge[:], v01[:], v23[:], op=mybir.AluOpType.is_ge)
            nc.vector.tensor_tensor(bestv[:], v01[:], v23[:], op=mybir.AluOpType.max)
            nc.vector.tensor_copy(besti[:], i23[:])
            nc.vector.copy_predicated(besti[:], ge[:], i01[:])

            # ---- critical path: gather this core's 64 quantized rows
            idx_scr = dpool.tile([B, 1], dt.float32)
            nc.sync.dma_start(idx_scr.rearrange("(p h) o -> p (h o)", p=P), besti[:])
            gidxf = mpool.tile([P, 1], dt.float32)
            nc.gpsimd.indirect_dma_start(
                out=gidxf[:], out_offset=None,
                in_=idx_scr[:],
                in_offset=bass.IndirectOffsetOnAxis(ap=mr[:, :1], axis=0),
            )
            base_f = mpool.tile([P, 1], dt.float32)
            nc.vector.tensor_scalar(base_f[:], gidxf[:], 8.0, scalar2=None, op0=mybir.AluOpType.mult)
            nc.vector.tensor_tensor(base_f[:], base_f[:], par[:], op=mybir.AluOpType.add)
            idx8 = mpool.tile([P, NSEG], dt.int32)
            for cseg in range(NSEG):
                nc.vector.tensor_scalar(
                    idx8[:, cseg:cseg + 1], base_f[:], float(cseg),
                    scalar2=None, op0=mybir.AluOpType.add,
                )
            for cseg in range(NSEG):
                gt = gpool.tile([P, SEG], dt.float32, tag="gt")
                nc.gpsimd.indirect_dma_start(
                    out=gt[:], out_offset=None,
                    in_=codebook[:],
                    in_offset=bass.IndirectOffsetOnAxis(ap=idx8[:, cseg:cseg + 1], axis=0),
                )
                nc.sync.dma_start(out_q[:, cseg * SEG:(cseg + 1) * SEG], gt[:])

            # ---- indices / min_distances (off critical path)
            idx_i = mpool.tile([P, 4], dt.int32)
            nc.vector.tensor_copy(idx_i[:], besti[:])
            nc.sync.dma_start(out_idx[:], idx_i[:])
            md = mpool.tile([P, 4], dt.float32)
            nc.scalar.activation(
                md[:], bestv[:], mybir.ActivationFunctionType.Sqrt, scale=-1.0,
            )
            nc.sync.dma_start(out_md[:], md[:])

            # ---- usage histogram (identical on every core, off critical path)
            idxf = mpool.tile([P, 4], dt.float32)
            nc.vector.tensor_copy(idxf[:], idx_i[:])
            iota_f = mpool.tile([P, K], dt.float32)
            nc.vector.tensor_copy(iota_f[:], iota_i[:])
            eqacc = mpool.tile([P, K], dt.float32)
            eqtmp = mpool.tile([P, K], dt.float32)
            for h in range(4):
                dst = eqacc if h == 0 else eqtmp
                nc.vector.tensor_tensor(
                    dst[:], idxf[:, h:h + 1].to_broadcast([P, K]), iota_f[:],
                    op=mybir.AluOpType.is_equal,
                )
                if h > 0:
                    nc.vector.tensor_tensor(eqacc[:], eqacc[:], eqtmp[:], op=mybir.AluOpType.add)
            ones1 = mpool.tile([P, 1], dt.float32)
            nc.vector.memset(ones1[:], 1.0)
            hist_ps = ppool.tile([1, K], dt.float32, space="PSUM")
            for half in range(2):
                nc.tensor.matmul(
                    hist_ps[:, half * 512:(half + 1) * 512],
                    ones1[:], eqacc[:, half * 512:(half + 1) * 512],
                    start=True, stop=True,
                )
            us = mpool.tile([1, K], dt.float32)
            nc.sync.dma_start(us[:], usage[:])
            nc.vector.tensor_tensor(us[:], us[:], hist_ps[:], op=mybir.AluOpType.add)
            nc.sync.dma_start(out_usage[:], us[:])

    nc.compile()
    return nc


def _get_nc():
    if "nc" not in _BUILT:
        _BUILT["nc"] = _build()
    return _BUILT["nc"]


def kernel(latents, codebook, usage_count):
    from concourse.bass_utils import run_bass_kernel_spmd

    latents = np.ascontiguousarray(np.asarray(latents, dtype=np.float32))
    codebook = np.ascontiguousarray(np.asarray(codebook, dtype=np.float32))
    usage_count = np.asarray(usage_count, dtype=np.float32)

    lat = latents.reshape(B, F)
    cb = codebook.reshape(K, F)

    # host-side norms (fp64 for accuracy, stored negated as fp32)
    x2 = (lat.astype(np.float64) ** 2).sum(axis=1)
    c2 = (cb.astype(np.float64) ** 2).sum(axis=1)

    # rearranged streaming operands: [128 p, 250 ch, cols], f index = ch*128 + p
    def rearr(mat2d, scale):
        # mat2d [rows, F] -> [P, NCH*rows] with layout [p, ch, row]
        m = (scale * mat2d).astype(np.float32)          # [rows, F]
        t = m.T.reshape(NCH, P, m.shape[0])             # [ch, p, row]
        return np.ascontiguousarray(t.transpose(1, 0, 2)).reshape(P, NCH * m.shape[0])

    lf_h = [rearr(lat[:, fh * FHALF:(fh + 1) * FHALF], 2.0) for fh in range(FH)]
    cf_hs = [
        [rearr(cb[s * KS:(s + 1) * KS, fh * FHALF:(fh + 1) * FHALF], 1.0) for s in range(C)]
        for fh in range(FH)
    ]

    cb4 = codebook.reshape(CB4_ROWS, SEG)
    usage_in = usage_count.reshape(1, K)
    parity = ((np.arange(P) % 2) * 4.0).astype(np.float32).reshape(P, 1)
    sbias_v = np.ascontiguousarray(
        np.broadcast_to(
            (np.arange(16) // 4 * KS).astype(np.float32), (P, 16)
        )
    )

    in_maps = []
    for c in range(NCORES):
        fh, s = divmod(c, C)
        nx = np.empty((P, MT), np.float32)
        for t in range(MT):
            nx[:, t] = -0.5 * x2[t * 128:(t + 1) * 128]
        ncn = np.ascontiguousarray(
            np.broadcast_to((-0.5 * c2[s * KS:(s + 1) * KS]).astype(np.float32), (P, KS))
        )
        rows = c * ROWS_PER_CORE + (np.arange(P) // 2)   # global row per partition (dup'd)
        mr = ((rows % P) * 4 + rows // P).astype(np.int32).reshape(P, 1)
        in_maps.append(dict(
            lf=lf_h[fh], cf=cf_hs[fh][s], normx=nx, normc=ncn,
            usage=usage_in, myrows2=mr, parity4=parity, sbias=sbias_v,
            codebook=cb4,
        ))

    nc = _get_nc()
    res = run_bass_kernel_spmd(
        nc, in_maps, core_ids=list(range(NCORES)), **_BUILT.get("run_kwargs", {})
    )
    _BUILT["last_result"] = res
    r0 = res.results[0]

    indices = r0["out_idx"].T.reshape(B).astype(np.int32)
    min_distances = r0["out_md"].T.reshape(B).astype(np.float32)
    new_usage = r0["out_usage"].reshape(K).astype(np.float32)
    quantized = np.concatenate(
        [res.results[c]["out_q"].reshape(ROWS_PER_CORE, F) for c in range(NCORES)], axis=0
    ).reshape((B,) + LATENT_SHAPE)
    return indices, quantized, min_distances, new_usage
